# revision 15
# baseline (speedup 1.0000x reference)
"""PointNet feature interpolation (3-NN inverse-distance) Trainium2 kernel.

Problem (per batch b of 8, one NeuronCore each):
  xyz1:    [3, N=8192]   source point coords
  xyz2:    [3, S=2048]   query point coords
  points1: [D=256, N]    source features
  out:     [D, S]        interpolated features

Device algorithm per core (v8):
  1. negdist[s, n] = 2*x2_s.x1_n - |x2_s|^2 - |x1_n|^2 as one K=12 bf16
     matmul (hh+hl+lh products per coord + n2h + n1h+n1l rows; abs err
     ~1e-4 — ample for block selection, exact values recomputed later).
  2. Scalar engine evacuates PSUM to bf16 SBUF; a pairwise tensor-max
     tree (DVE 2x bf16) folds 8192 -> 512 block maxima (block j = points
     {j + 512m}); vector.max/max_index give the top-8 blocks per row.
  3. Top-NB=5 blocks' point data ([2x,2y,2z,n1] f32, 256B rows) gathered
     by gpsimd.dma_gather in 1024-index pieces (the SWDGE descriptor
     ring holds 1024 descriptors); the int16 wrapped index layout is
     built on device via one DRAM round trip per half.
  4. Exact fp32 candidate re-distances (DVE); DVE clears the low 7
     mantissa bits and ORs in the candidate slot id; vector.max then
     yields top-3 values WITH slots embedded; a 5-wide equality match
     recovers the block id; w_k = (1/(d_k+1e-8))/sum.
  5. Features gathered by dma_gather pieces from p1t; weighted sum
     (scalar muls + DVE fused mul-add); DMA out as [S, D]; host
     transposes.
  Work is split in two chunk-halves so gathers/decode of half 0 overlap
  pass-1 of half 1.
"""

import numpy as np
import ml_dtypes

B, N, S, D = 8, 8192, 2048, 256
P = 128
NCHUNK = S // P      # 16 query-row chunks per core
NT = 512             # matmul moving free dim (one PSUM bank)
K = 12               # contraction rows of the distance matmul
NBLK = 512           # blocks per row (block j = points {j + 512m})
BPTS = N // NBLK     # 16 points per block
NB = 5               # candidate blocks kept per row
NCAND = NB * BPTS    # 80 candidate points per row
HC = NCHUNK // 2     # chunks per half
NIXH = P * HC * NB   # xblk gather indices per half (5120)
NIFH = P * HC * 3    # feature gather indices per half (3072)
GP = 1024            # gather piece size (SWDGE ring capacity)

MASK_KEEP = 0xFFFFFF80   # clear low 7 mantissa bits (slot id space)
MASK_SLOT = 0x7F

_COMPILED = None


def _build_bass(abl=()):
    import concourse.bass as bass
    import concourse.mybir as mybir
    import concourse.tile as tile
    from concourse import bacc

    f32 = mybir.dt.float32
    bf16 = mybir.dt.bfloat16
    u32 = mybir.dt.uint32
    i16 = mybir.dt.int16
    Alu = mybir.AluOpType
    X = mybir.AxisListType.X

    nc = bacc.Bacc(None)
    x2m = nc.dram_tensor("x2m", [K, S], bf16, kind="ExternalInput")
    x1m = nc.dram_tensor("x1m", [K, N], bf16, kind="ExternalInput")
    p1t = nc.dram_tensor("p1t", [N, D], bf16, kind="ExternalInput")
    xblk = nc.dram_tensor("xblk", [NBLK, BPTS * 4], f32, kind="ExternalInput")
    x2n = nc.dram_tensor("x2n", [P, NCHUNK, 4], f32, kind="ExternalInput")
    scr_x = nc.dram_tensor("scr_x", [2 * NIXH], i16, kind="Internal")
    scr_f = nc.dram_tensor("scr_f", [2 * NIFH], i16, kind="Internal")
    outS = nc.dram_tensor("outS", [S, D], f32, kind="ExternalOutput")

    with tile.TileContext(nc) as tc:
        with (
            tc.tile_pool(name="const", bufs=1) as cpool,
            tc.tile_pool(name="negb", bufs=2) as nbpool,
            tc.tile_pool(name="tree", bufs=2) as tpool,
            tc.tile_pool(name="mm", bufs=2, space="PSUM") as mmpool,
            tc.tile_pool(name="small", bufs=4) as spool,
            tc.tile_pool(name="gat", bufs=4) as gpool,
            tc.tile_pool(name="persist", bufs=1) as ppool,
        ):
            x2s = cpool.tile([K, S], bf16)
            nc.sync.dma_start(x2s[:], x2m[:])
            x1s = cpool.tile([K, N], bf16)
            nc.sync.dma_start(x1s[:], x1m[:])
            x2n_sb = cpool.tile([P, NCHUNK, 4], f32)
            nc.sync.dma_start(x2n_sb[:], x2n[:])
            # slot iota: iot[p, k, m] = BPTS*k + m  (slot id in enc)
            iot = cpool.tile([P, NB, BPTS], u32)
            nc.gpsimd.iota(iot[:], pattern=[[BPTS, NB], [1, BPTS]],
                           base=0, channel_multiplier=0)
            iota5 = cpool.tile([P, NB], u32)
            nc.gpsimd.iota(iota5[:], pattern=[[1, NB]],
                           base=0, channel_multiplier=0)

            bi_all = ppool.tile([P, NCHUNK, 8], u32, tag="bi")
            gxb = ppool.tile([P, NCHUNK * NB, BPTS * 4], f32, tag="gxb")
            negdc = ppool.tile([P, NCHUNK, NB, BPTS], f32, tag="negdc")
            encu = ppool.tile([P, NCHUNK, NB, BPTS], u32, tag="encu")
            cv8 = ppool.tile([P, NCHUNK, 8], f32, tag="cv8")
            w3 = ppool.tile([P, NCHUNK, 3], f32, tag="w3")
            n3 = ppool.tile([P, NCHUNK, 3], u32, tag="n3")
            gfeat = ppool.tile([P, NCHUNK * 3, D], bf16, tag="gfeat")

            def pass1_chunk(ci):
                negb = nbpool.tile([P, N], bf16, tag="negb")
                for g in range(4):
                    ps = mmpool.tile([P, 4 * NT], f32, tag="mm")
                    if "nomm" in abl:
                        nc.vector.memset(ps[:, 0:8], 1.0)
                    else:
                        for j in range(4):
                            nt = g * 4 + j
                            nc.tensor.matmul(
                                ps[:, j * NT:(j + 1) * NT],
                                lhsT=x2s[:, ci * P:(ci + 1) * P],
                                rhs=x1s[:, nt * NT:(nt + 1) * NT],
                                start=True,
                                stop=True,
                            )
                    nc.scalar.copy(negb[:, g * 2048:(g + 1) * 2048], ps[:])

                # pairwise-max tree 8192 -> 512 (bf16 2x mode)
                bm = tpool.tile([P, NBLK], bf16, tag="bm")
                tsc = tpool.tile([P, 4096], bf16, tag="tree")
                nc.vector.tensor_tensor(
                    out=tsc[:], in0=negb[:, 0:4096], in1=negb[:, 4096:8192],
                    op=Alu.max)
                nc.vector.tensor_tensor(
                    out=tsc[:, 0:2048], in0=tsc[:, 0:2048],
                    in1=tsc[:, 2048:4096], op=Alu.max)
                nc.vector.tensor_tensor(
                    out=tsc[:, 0:1024], in0=tsc[:, 0:1024],
                    in1=tsc[:, 1024:2048], op=Alu.max)
                nc.vector.tensor_tensor(
                    out=bm[:], in0=tsc[:, 0:512], in1=tsc[:, 512:1024],
                    op=Alu.max)

                bv8 = spool.tile([P, 8], bf16, tag="bv8")
                nc.vector.max(out=bv8[:], in_=bm[:])
                nc.vector.max_index(out=bi_all[:, ci], in_max=bv8[:],
                                    in_values=bm[:])

            def xblk_gather_half(h):
                """idx prep + gather pieces for chunks [h*HC, (h+1)*HC)."""
                bi16 = ppool.tile([P, HC * NB], i16, tag=f"bi16_{h}")
                nc.vector.tensor_copy(
                    bi16[:].rearrange("p (c k) -> p c k", k=NB),
                    bi_all[:, h * HC:(h + 1) * HC, 0:NB])
                nc.sync.dma_start(
                    scr_x[h * NIXH:(h + 1) * NIXH].rearrange(
                        "(j p) -> p j", p=P),
                    bi16[:])
                iwx = ppool.tile([P, NIXH // 16], i16, tag=f"iwx{h}")
                for g in range(8):
                    nc.sync.dma_start(
                        iwx[16 * g:16 * (g + 1)].rearrange(
                            "l (a b) -> l a b", b=8),
                        scr_x[h * NIXH:(h + 1) * NIXH].rearrange(
                            "(a b l) -> l a b", b=8, l=16))
                j0 = h * HC * NB
                for t in range(NIXH // GP):
                    if "nogather" in abl:
                        nc.sync.dma_start(
                            gxb[:, j0 + t * 8:j0 + (t + 1) * 8],
                            xblk[0:P, :].unsqueeze(1).to_broadcast(
                                [P, 8, BPTS * 4]))
                    else:
                        nc.gpsimd.dma_gather(
                            out_ap=gxb[:, j0 + t * 8:j0 + (t + 1) * 8],
                            in_ap=xblk[:],
                            idxs_ap=iwx[:, t * (GP // 16):
                                        (t + 1) * (GP // 16)],
                            num_idxs=GP,
                            num_idxs_reg=GP,
                            elem_size=BPTS * 4,
                        )

            def candidates_half(h):
                """exact distances, enc, top-3, decode, weights."""
                cs = slice(h * HC, (h + 1) * HC)
                gv = gxb[:, h * HC * NB:(h + 1) * HC * NB].rearrange(
                    "p (c k) (m f) -> p c k m f", k=NB, f=4)
                sh = [P, HC, NB, BPTS]

                def x2c(c):
                    return (x2n_sb[:, cs, c:c + 1].unsqueeze(3)
                            .to_broadcast(sh))

                ev = nc.vector
                tmp = ppool.tile(sh, f32, tag=f"tmpA{h}")
                tmp2 = ppool.tile(sh, f32, tag=f"tmpB{h}")
                ev.tensor_tensor(out=tmp[:], in0=gv[:, :, :, :, 0],
                                 in1=x2c(0), op=Alu.mult)
                ev.tensor_tensor(out=tmp2[:], in0=gv[:, :, :, :, 1],
                                 in1=x2c(1), op=Alu.mult)
                ev.tensor_tensor(out=tmp[:], in0=tmp[:], in1=tmp2[:],
                                 op=Alu.add)
                ev.tensor_tensor(out=tmp2[:], in0=gv[:, :, :, :, 2],
                                 in1=x2c(2), op=Alu.mult)
                ev.tensor_tensor(out=tmp[:], in0=tmp[:], in1=tmp2[:],
                                 op=Alu.add)
                ev.tensor_tensor(out=tmp[:], in0=tmp[:],
                                 in1=gv[:, :, :, :, 3], op=Alu.subtract)
                ev.tensor_tensor(out=negdc[:, cs], in0=tmp[:], in1=x2c(3),
                                 op=Alu.subtract)

                nc.vector.tensor_scalar(out=encu[:, cs],
                                        in0=negdc[:, cs].bitcast(u32),
                                        scalar1=MASK_KEEP, scalar2=None,
                                        op0=Alu.bitwise_and)
                nc.vector.tensor_tensor(
                    out=encu[:, cs], in0=encu[:, cs],
                    in1=iot[:].unsqueeze(1).to_broadcast(sh),
                    op=Alu.bitwise_or)
                encf = encu[:].bitcast(f32)
                for ci in range(h * HC, (h + 1) * HC):
                    nc.vector.max(out=cv8[:, ci], in_=encf[:, ci])

                # decode: slot -> (block k, point m) -> global index
                selu = cv8[:].bitcast(u32)[:, cs, 0:3]       # [P, HC, 3]
                s3 = ppool.tile([P, HC, 3], u32, tag=f"s3_{h}")
                nc.vector.tensor_scalar(out=s3[:], in0=selu,
                                        scalar1=MASK_SLOT, scalar2=None,
                                        op0=Alu.bitwise_and)
                k3 = ppool.tile([P, HC, 3], u32, tag=f"k3_{h}")
                nc.vector.tensor_scalar(out=k3[:], in0=s3[:], scalar1=4,
                                        scalar2=None,
                                        op0=Alu.logical_shift_right)
                m3 = ppool.tile([P, HC, 3], u32, tag=f"m3_{h}")
                nc.vector.tensor_scalar(out=m3[:], in0=s3[:], scalar1=0xF,
                                        scalar2=None, op0=Alu.bitwise_and)
                shq = [P, HC, 3, NB]
                eqk = ppool.tile(shq, u32, tag=f"eqk{h}")
                nc.vector.tensor_tensor(
                    out=eqk[:],
                    in0=k3[:].unsqueeze(3).to_broadcast(shq),
                    in1=iota5[:].unsqueeze(1).unsqueeze(2).to_broadcast(shq),
                    op=Alu.is_equal)
                nc.vector.tensor_tensor(
                    out=eqk[:], in0=eqk[:],
                    in1=bi_all[:, cs, 0:NB].unsqueeze(2).to_broadcast(shq),
                    op=Alu.mult)
                bik = ppool.tile([P, HC, 3], u32, tag=f"bik{h}")
                nc.vector.tensor_reduce(out=bik[:], in_=eqk[:], axis=X,
                                        op=Alu.max)
                # global index n3 = bik + 512 * m3
                nc.vector.scalar_tensor_tensor(
                    out=n3[:, cs], in0=m3[:], scalar=NBLK, in1=bik[:],
                    op0=Alu.mult, op1=Alu.add)

                d3u = ppool.tile([P, HC, 3], u32, tag=f"d3u{h}")
                nc.vector.tensor_scalar(out=d3u[:], in0=selu,
                                        scalar1=MASK_KEEP, scalar2=None,
                                        op0=Alu.bitwise_and)
                d3 = ppool.tile([P, HC, 3], f32, tag=f"d3_{h}")
                nc.vector.tensor_scalar(out=d3[:], in0=d3u[:].bitcast(f32),
                                        scalar1=-1.0, scalar2=1e-8,
                                        op0=Alu.mult, op1=Alu.add)
                nc.vector.reciprocal(d3[:], d3[:])
                rsum = ppool.tile([P, HC], f32, tag=f"rsum{h}")
                nc.vector.tensor_reduce(out=rsum[:], in_=d3[:], axis=X,
                                        op=Alu.add)
                nc.vector.reciprocal(rsum[:], rsum[:])
                nc.vector.tensor_tensor(
                    out=w3[:, cs], in0=d3[:],
                    in1=rsum[:].unsqueeze(2).to_broadcast([P, HC, 3]),
                    op=Alu.mult)

            def feat_gather_half(h):
                n316 = ppool.tile([P, HC * 3], i16, tag=f"n316_{h}")
                nc.vector.tensor_copy(
                    n316[:].rearrange("p (c k) -> p c k", k=3),
                    n3[:, h * HC:(h + 1) * HC])
                nc.sync.dma_start(
                    scr_f[h * NIFH:(h + 1) * NIFH].rearrange(
                        "(j p) -> p j", p=P),
                    n316[:])
                iwf = ppool.tile([P, NIFH // 16], i16, tag=f"iwf{h}")
                for g in range(8):
                    nc.sync.dma_start(
                        iwf[16 * g:16 * (g + 1)].rearrange(
                            "l (a b) -> l a b", b=8),
                        scr_f[h * NIFH:(h + 1) * NIFH].rearrange(
                            "(a b l) -> l a b", b=8, l=16))
                j0 = h * HC * 3
                for t in range(NIFH // GP):
                    if "nofgather" in abl:
                        nc.sync.dma_start(
                            gfeat[:, j0 + t * 8:j0 + (t + 1) * 8],
                            p1t[0:P, :].unsqueeze(1).to_broadcast(
                                [P, 8, D]))
                    else:
                        nc.gpsimd.dma_gather(
                            out_ap=gfeat[:, j0 + t * 8:j0 + (t + 1) * 8],
                            in_ap=p1t[:],
                            idxs_ap=iwf[:, t * (GP // 16):
                                        (t + 1) * (GP // 16)],
                            num_idxs=GP,
                            num_idxs_reg=GP,
                            elem_size=D,
                        )

            def interp_half(h):
                for ci in range(h * HC, (h + 1) * HC):
                    acc = gpool.tile([P, D], f32, tag="acc")
                    acc2 = gpool.tile([P, D], f32, tag="acc2")
                    gm2 = gpool.tile([P, D], f32, tag="gm2")
                    nc.scalar.mul(acc[:], gfeat[:, ci * 3], w3[:, ci, 0:1])
                    nc.vector.scalar_tensor_tensor(
                        out=acc[:], in0=gfeat[:, ci * 3 + 1],
                        scalar=w3[:, ci, 1:2], in1=acc[:],
                        op0=Alu.mult, op1=Alu.add)
                    nc.scalar.mul(gm2[:], gfeat[:, ci * 3 + 2],
                                  w3[:, ci, 2:3])
                    nc.vector.tensor_add(acc2[:], acc[:], gm2[:])
                    nc.sync.dma_start(outS[ci * P:(ci + 1) * P, :], acc2[:])

            # ---- schedule: halves pipeline across engines ----
            for h in range(2):
                for ci in range(h * HC, (h + 1) * HC):
                    pass1_chunk(ci)
                xblk_gather_half(h)
                candidates_half(h)
                feat_gather_half(h)
                interp_half(h)

    nc.finalize()
    return nc


def _split2(x):
    """Split fp64 array into 2 bf16 terms h+l ~ x (residual ~2^-18|x|)."""
    bf = ml_dtypes.bfloat16
    h = x.astype(bf)
    l = (x - h.astype(np.float64)).astype(bf)
    return h, l


def _host_matrices(xyz2b, xyz1b):
    """Build the K=12 bf16 contraction matrices for one batch.

    negdist[s, n] = sum_k X2[k, s] * X1[k, n]
                  ~ 2 x2_s.x1_n - |x2_s|^2 - |x1_n|^2
    """
    bf = ml_dtypes.bfloat16
    x2 = xyz2b.astype(np.float64)   # [3, S]
    x1 = xyz1b.astype(np.float64)   # [3, N]
    n2 = (x2 * x2).sum(axis=0)      # [S]
    n1 = (x1 * x1).sum(axis=0)      # [N]

    Srows, Nrows = [], []
    for c in range(3):
        qh, ql = _split2(2.0 * x2[c])
        ph, pl = _split2(x1[c])
        for a, b_ in ((qh, ph), (qh, pl), (ql, ph)):
            Srows.append(a)
            Nrows.append(b_)
    ones_s = np.ones(x2.shape[1], dtype=bf)
    ones_n = np.ones(x1.shape[1], dtype=bf)
    n2h, _ = _split2(-n2)
    Srows.append(n2h)
    Nrows.append(ones_n)
    for t in _split2(-n1):
        Srows.append(ones_s)
        Nrows.append(t)
    X2 = np.stack([np.asarray(r, dtype=bf) for r in Srows])   # [12, S]
    X1 = np.stack([np.asarray(r, dtype=bf) for r in Nrows])   # [12, N]
    return X2, X1, n2.astype(np.float32), n1.astype(np.float32)


def _prep_inputs(xyz1, xyz2, points1):
    xyz1 = np.asarray(xyz1, dtype=np.float32)
    xyz2 = np.asarray(xyz2, dtype=np.float32)
    points1 = np.asarray(points1, dtype=np.float32)
    in_maps = []
    for b in range(B):
        X2, X1, n2, n1 = _host_matrices(xyz2[b], xyz1[b])
        p1tb = np.ascontiguousarray(points1[b].T).astype(
            ml_dtypes.bfloat16)  # [N, D] bf16
        # block table: row j holds points {j + 512m}, each [2x,2y,2z,n1]
        xb = np.empty((NBLK, BPTS, 4), dtype=np.float32)
        pts = (2.0 * xyz1[b].T).reshape(BPTS, NBLK, 3)   # [m, j, 3]
        xb[:, :, 0:3] = pts.transpose(1, 0, 2)
        xb[:, :, 3] = n1.reshape(BPTS, NBLK).T
        # per-query [x, y, z, n2], laid out [p, chunk, 4]
        xq = np.empty((P, NCHUNK, 4), dtype=np.float32)
        q = xyz2[b].T.reshape(NCHUNK, P, 3)        # [chunk, p, 3]
        xq[:, :, 0:3] = q.transpose(1, 0, 2)
        xq[:, :, 3] = n2.reshape(NCHUNK, P).T
        in_maps.append({
            "x2m": X2, "x1m": X1, "p1t": p1tb,
            "xblk": xb.reshape(NBLK, BPTS * 4), "x2n": xq,
        })
    return in_maps


def _get_compiled():
    global _COMPILED
    if _COMPILED is None:
        _COMPILED = _build_bass()
    return _COMPILED


def kernel(xyz1, xyz2, points1):
    from concourse.bass_utils import run_bass_kernel_spmd

    nc = _get_compiled()
    in_maps = _prep_inputs(xyz1, xyz2, points1)
    res = run_bass_kernel_spmd(nc, in_maps, core_ids=list(range(B)))
    out = np.stack([r["outS"] for r in res.results])     # [B, S, D]
    return np.ascontiguousarray(out.transpose(0, 2, 1)).astype(np.float32)


if __name__ == "__main__":
    rng = np.random.default_rng(0)
    xyz1 = rng.standard_normal((B, 3, N), dtype=np.float32)
    xyz2 = rng.standard_normal((B, 3, S), dtype=np.float32)
    p1 = rng.standard_normal((B, D, N), dtype=np.float32)
    out = kernel(xyz1, xyz2, p1)
    print("out", out.shape, out.dtype)


# revision 19
# speedup vs baseline: 1.4087x; 1.4087x over previous
"""PointNet feature interpolation (3-NN inverse-distance) Trainium2 kernel.

Problem (per batch b of 8, one NeuronCore each):
  xyz1:    [3, N=8192]   source point coords
  xyz2:    [3, S=2048]   query point coords
  points1: [D=256, N]    source features
  out:     [D, S]        interpolated features

Device algorithm per core (v9):
  1. negdist[s, n] = 2*x2_s.x1_n - |x2_s|^2 - |x1_n|^2 as one K=12 bf16
     matmul (hh+hl+lh products per coord + n2h + n1h+n1l rows; abs err
     ~1e-4 — ample for block selection, exact values recomputed later).
  2. Scalar engine evacuates PSUM to bf16 SBUF; a pairwise tensor-max
     tree (DVE 2x bf16) folds 8192 -> 512 block maxima (block j = points
     {j + 512m}); vector.max/max_index give the top-8 blocks per row.
  3. Top-NB=5 blocks' point data ([2x,2y,2z,n1] f32, 256B rows) gathered
     by gpsimd.dma_gather in <=1024-index pieces (SWDGE ring capacity);
     the int16 wrapped index layout is built per 4-chunk quarter via a
     DRAM round trip with contiguous-chunk access patterns.
  4. Exact fp32 candidate re-distances (DVE); DVE clears the low 7
     mantissa bits and ORs in the candidate slot id; vector.max then
     yields top-3 values WITH slots embedded; a 5-wide equality match
     recovers the block id; w_k = (1/(d_k+1e-8))/sum.
  5. Features gathered by dma_gather pieces from p1t; weighted sum
     (scalar muls + DVE fused mul-add); DMA out as [S, D]; host
     transposes.
  Stages are emitted interleaved across quarters so every engine queue
  stays busy (no head-of-line blocking).
"""

import numpy as np
import ml_dtypes

B, N, S, D = 8, 8192, 2048, 256
P = 128
NCHUNK = S // P      # 16 query-row chunks per core
NT = 512             # matmul moving free dim (one PSUM bank)
K = 12               # contraction rows of the distance matmul
NBLK = 512           # blocks per row (block j = points {j + 512m})
BPTS = N // NBLK     # 16 points per block
NB = 5               # candidate blocks kept per row
NCAND = NB * BPTS    # 80 candidate points per row
NQ = 4               # quarters
QC = NCHUNK // NQ    # chunks per quarter
NIXQ = P * QC * NB   # xblk gather indices per quarter (2560)
NIFQ = P * QC * 3    # feature gather indices per quarter (1536)
GP = 1024            # max gather piece (SWDGE ring capacity)

MASK_KEEP = 0xFFFFFF80   # clear low 7 mantissa bits (slot id space)
MASK_SLOT = 0x7F

_COMPILED = None


def _pieces(total):
    out = []
    o = 0
    while o < total:
        out.append((o, min(GP, total - o)))
        o += min(GP, total - o)
    return out


def _build_bass(abl=()):
    import concourse.bass as bass
    import concourse.mybir as mybir
    import concourse.tile as tile
    from concourse import bacc

    f32 = mybir.dt.float32
    bf16 = mybir.dt.bfloat16
    u32 = mybir.dt.uint32
    i16 = mybir.dt.int16
    Alu = mybir.AluOpType
    X = mybir.AxisListType.X

    nc = bacc.Bacc(None)
    x2m = nc.dram_tensor("x2m", [K, S], bf16, kind="ExternalInput")
    x1m = nc.dram_tensor("x1m", [K, N], bf16, kind="ExternalInput")
    p1t = nc.dram_tensor("p1t", [N, D], bf16, kind="ExternalInput")
    xblk = nc.dram_tensor("xblk", [NBLK, BPTS * 4], f32, kind="ExternalInput")
    x2n = nc.dram_tensor("x2n", [P, NCHUNK, 4], f32, kind="ExternalInput")
    scr_x = nc.dram_tensor("scr_x", [NQ * NIXQ], i16, kind="Internal")
    scr_f = nc.dram_tensor("scr_f", [NQ * NIFQ], i16, kind="Internal")
    outS = nc.dram_tensor("outS", [S, D], f32, kind="ExternalOutput")

    with tile.TileContext(nc) as tc:
        with (
            tc.tile_pool(name="const", bufs=1) as cpool,
            tc.tile_pool(name="negb", bufs=2) as nbpool,
            tc.tile_pool(name="tree", bufs=2) as tpool,
            tc.tile_pool(name="mm", bufs=2, space="PSUM") as mmpool,
            tc.tile_pool(name="small", bufs=4) as spool,
            tc.tile_pool(name="gat", bufs=4) as gpool,
            tc.tile_pool(name="persist", bufs=1) as ppool,
        ):
            x2s = cpool.tile([K, S], bf16)
            nc.sync.dma_start(x2s[:], x2m[:])
            x1s = cpool.tile([K, N], bf16)
            nc.sync.dma_start(x1s[:], x1m[:])
            x2n_sb = cpool.tile([P, NCHUNK, 4], f32)
            nc.sync.dma_start(x2n_sb[:], x2n[:])
            # slot iota: iot[p, k, m] = BPTS*k + m  (slot id in enc)
            iot = cpool.tile([P, NB, BPTS], u32)
            nc.gpsimd.iota(iot[:], pattern=[[BPTS, NB], [1, BPTS]],
                           base=0, channel_multiplier=0)
            iota5 = cpool.tile([P, NB], u32)
            nc.gpsimd.iota(iota5[:], pattern=[[1, NB]],
                           base=0, channel_multiplier=0)

            bi_all = ppool.tile([P, NCHUNK, 8], u32, tag="bi")
            gxb = ppool.tile([P, NCHUNK * NB, BPTS * 4], f32, tag="gxb")
            negdc = ppool.tile([P, NCHUNK, NB, BPTS], f32, tag="negdc")
            encu = ppool.tile([P, NCHUNK, NB, BPTS], u32, tag="encu")
            cv8 = ppool.tile([P, NCHUNK, 8], f32, tag="cv8")
            w3 = ppool.tile([P, NCHUNK, 3], f32, tag="w3")
            n3 = ppool.tile([P, NCHUNK, 3], u32, tag="n3")
            gfeat = ppool.tile([P, NCHUNK * 3, D], bf16, tag="gfeat")

            def pass1_chunk(ci):
                negb = nbpool.tile([P, N], bf16, tag="negb")
                for g in range(4):
                    ps = mmpool.tile([P, 4 * NT], f32, tag="mm")
                    if "nomm" in abl:
                        nc.vector.memset(ps[:, 0:8], 1.0)
                    else:
                        for j in range(4):
                            nt = g * 4 + j
                            nc.tensor.matmul(
                                ps[:, j * NT:(j + 1) * NT],
                                lhsT=x2s[:, ci * P:(ci + 1) * P],
                                rhs=x1s[:, nt * NT:(nt + 1) * NT],
                                start=True,
                                stop=True,
                            )
                    nc.scalar.copy(negb[:, g * 2048:(g + 1) * 2048], ps[:])

                # pairwise-max tree 8192 -> 512 (bf16 2x mode)
                bm = tpool.tile([P, NBLK], bf16, tag="bm")
                tsc = tpool.tile([P, 4096], bf16, tag="tree")
                nc.vector.tensor_tensor(
                    out=tsc[:], in0=negb[:, 0:4096], in1=negb[:, 4096:8192],
                    op=Alu.max)
                nc.vector.tensor_tensor(
                    out=tsc[:, 0:2048], in0=tsc[:, 0:2048],
                    in1=tsc[:, 2048:4096], op=Alu.max)
                nc.vector.tensor_tensor(
                    out=tsc[:, 0:1024], in0=tsc[:, 0:1024],
                    in1=tsc[:, 1024:2048], op=Alu.max)
                nc.vector.tensor_tensor(
                    out=bm[:], in0=tsc[:, 0:512], in1=tsc[:, 512:1024],
                    op=Alu.max)

                bv8 = spool.tile([P, 8], bf16, tag="bv8")
                nc.vector.max(out=bv8[:], in_=bm[:])
                nc.vector.max_index(out=bi_all[:, ci], in_max=bv8[:],
                                    in_values=bm[:])

            def idx_prep(q, src_slice_fn, scr, NI, JW, tagp):
                """Build wrapped int16 idxs for quarter q.

                D1[p, j] = SRC[p, j] (row-major in DRAM); readback
                iw[16g+l, (j, b)] = D1[16b+l, j] replicated over g.
                """
                s16 = ppool.tile([P, JW], i16, tag=f"{tagp}s{q}")
                src, kk = src_slice_fn()
                nc.vector.tensor_copy(
                    s16[:].rearrange("p (c k) -> p c k", k=kk), src)
                nc.sync.dma_start(
                    scr[q * NI:(q + 1) * NI].rearrange("(p j) -> p j", p=P),
                    s16[:])
                iw = ppool.tile([P, NI // 16], i16, tag=f"{tagp}w{q}")
                for g in range(8):
                    nc.sync.dma_start(
                        iw[16 * g:16 * (g + 1)].rearrange(
                            "l (j b) -> l j b", b=8),
                        scr[q * NI:(q + 1) * NI].rearrange(
                            "(b l j) -> l j b", b=8, l=16))
                return iw

            def xblk_prep(q):
                return idx_prep(
                    q,
                    lambda: (bi_all[:, q * QC:(q + 1) * QC, 0:NB], NB),
                    scr_x, NIXQ, QC * NB, "ix")

            def xblk_gather(q, iw):
                j0 = q * QC * NB
                for (o, ln) in _pieces(NIXQ):
                    if "nogather" in abl:
                        nc.sync.dma_start(
                            gxb[:, j0 + o // P:j0 + (o + ln) // P],
                            xblk[0:P, :].unsqueeze(1).to_broadcast(
                                [P, ln // P, BPTS * 4]))
                    else:
                        nc.gpsimd.dma_gather(
                            out_ap=gxb[:, j0 + o // P:j0 + (o + ln) // P],
                            in_ap=xblk[:],
                            idxs_ap=iw[:, o // 16:(o + ln) // 16],
                            num_idxs=ln,
                            num_idxs_reg=ln,
                            elem_size=BPTS * 4,
                        )

            def candidates_q(q):
                cs = slice(q * QC, (q + 1) * QC)
                gv = gxb[:, q * QC * NB:(q + 1) * QC * NB].rearrange(
                    "p (c k) (m f) -> p c k m f", k=NB, f=4)
                sh = [P, QC, NB, BPTS]

                def x2c(c):
                    return (x2n_sb[:, cs, c:c + 1].unsqueeze(3)
                            .to_broadcast(sh))

                ev = nc.vector
                tmp = ppool.tile(sh, f32, tag=f"tmpA{q}")
                tmp2 = ppool.tile(sh, f32, tag=f"tmpB{q}")
                ev.tensor_tensor(out=tmp[:], in0=gv[:, :, :, :, 0],
                                 in1=x2c(0), op=Alu.mult)
                ev.tensor_tensor(out=tmp2[:], in0=gv[:, :, :, :, 1],
                                 in1=x2c(1), op=Alu.mult)
                ev.tensor_tensor(out=tmp[:], in0=tmp[:], in1=tmp2[:],
                                 op=Alu.add)
                ev.tensor_tensor(out=tmp2[:], in0=gv[:, :, :, :, 2],
                                 in1=x2c(2), op=Alu.mult)
                ev.tensor_tensor(out=tmp[:], in0=tmp[:], in1=tmp2[:],
                                 op=Alu.add)
                ev.tensor_tensor(out=tmp[:], in0=tmp[:],
                                 in1=gv[:, :, :, :, 3], op=Alu.subtract)
                ev.tensor_tensor(out=negdc[:, cs], in0=tmp[:], in1=x2c(3),
                                 op=Alu.subtract)

                nc.vector.tensor_scalar(out=encu[:, cs],
                                        in0=negdc[:, cs].bitcast(u32),
                                        scalar1=MASK_KEEP, scalar2=None,
                                        op0=Alu.bitwise_and)
                nc.vector.tensor_tensor(
                    out=encu[:, cs], in0=encu[:, cs],
                    in1=iot[:].unsqueeze(1).to_broadcast(sh),
                    op=Alu.bitwise_or)
                encf = encu[:].bitcast(f32)
                for ci in range(q * QC, (q + 1) * QC):
                    nc.vector.max(out=cv8[:, ci], in_=encf[:, ci])

                # decode: slot -> (block k, point m) -> global index
                selu = cv8[:].bitcast(u32)[:, cs, 0:3]       # [P, QC, 3]
                s3 = ppool.tile([P, QC, 3], u32, tag=f"s3_{q}")
                nc.vector.tensor_scalar(out=s3[:], in0=selu,
                                        scalar1=MASK_SLOT, scalar2=None,
                                        op0=Alu.bitwise_and)
                k3 = ppool.tile([P, QC, 3], u32, tag=f"k3_{q}")
                nc.vector.tensor_scalar(out=k3[:], in0=s3[:], scalar1=4,
                                        scalar2=None,
                                        op0=Alu.logical_shift_right)
                m3 = ppool.tile([P, QC, 3], u32, tag=f"m3_{q}")
                nc.vector.tensor_scalar(out=m3[:], in0=s3[:], scalar1=0xF,
                                        scalar2=None, op0=Alu.bitwise_and)
                shq = [P, QC, 3, NB]
                eqk = ppool.tile(shq, u32, tag=f"eqk{q}")
                nc.vector.tensor_tensor(
                    out=eqk[:],
                    in0=k3[:].unsqueeze(3).to_broadcast(shq),
                    in1=iota5[:].unsqueeze(1).unsqueeze(2).to_broadcast(shq),
                    op=Alu.is_equal)
                nc.vector.tensor_tensor(
                    out=eqk[:], in0=eqk[:],
                    in1=bi_all[:, cs, 0:NB].unsqueeze(2).to_broadcast(shq),
                    op=Alu.mult)
                bik = ppool.tile([P, QC, 3], u32, tag=f"bik{q}")
                nc.vector.tensor_reduce(out=bik[:], in_=eqk[:], axis=X,
                                        op=Alu.max)
                # global index n3 = bik + 512 * m3
                nc.vector.scalar_tensor_tensor(
                    out=n3[:, cs], in0=m3[:], scalar=NBLK, in1=bik[:],
                    op0=Alu.mult, op1=Alu.add)

                d3u = ppool.tile([P, QC, 3], u32, tag=f"d3u{q}")
                nc.vector.tensor_scalar(out=d3u[:], in0=selu,
                                        scalar1=MASK_KEEP, scalar2=None,
                                        op0=Alu.bitwise_and)
                d3 = ppool.tile([P, QC, 3], f32, tag=f"d3_{q}")
                nc.vector.tensor_scalar(out=d3[:], in0=d3u[:].bitcast(f32),
                                        scalar1=-1.0, scalar2=1e-8,
                                        op0=Alu.mult, op1=Alu.add)
                nc.vector.reciprocal(d3[:], d3[:])
                rsum = ppool.tile([P, QC], f32, tag=f"rsum{q}")
                nc.vector.tensor_reduce(out=rsum[:], in_=d3[:], axis=X,
                                        op=Alu.add)
                nc.vector.reciprocal(rsum[:], rsum[:])
                nc.vector.tensor_tensor(
                    out=w3[:, cs], in0=d3[:],
                    in1=rsum[:].unsqueeze(2).to_broadcast([P, QC, 3]),
                    op=Alu.mult)

            def feat_prep(q):
                return idx_prep(
                    q,
                    lambda: (n3[:, q * QC:(q + 1) * QC], 3),
                    scr_f, NIFQ, QC * 3, "if")

            def feat_gather(q, iw):
                j0 = q * QC * 3
                for (o, ln) in _pieces(NIFQ):
                    if "nofgather" in abl:
                        nc.sync.dma_start(
                            gfeat[:, j0 + o // P:j0 + (o + ln) // P],
                            p1t[0:P, :].unsqueeze(1).to_broadcast(
                                [P, ln // P, D]))
                    else:
                        nc.gpsimd.dma_gather(
                            out_ap=gfeat[:, j0 + o // P:j0 + (o + ln) // P],
                            in_ap=p1t[:],
                            idxs_ap=iw[:, o // 16:(o + ln) // 16],
                            num_idxs=ln,
                            num_idxs_reg=ln,
                            elem_size=D,
                        )

            def interp_q(q):
                for ci in range(q * QC, (q + 1) * QC):
                    acc = gpool.tile([P, D], f32, tag="acc")
                    acc2 = gpool.tile([P, D], f32, tag="acc2")
                    gm2 = gpool.tile([P, D], f32, tag="gm2")
                    nc.scalar.mul(acc[:], gfeat[:, ci * 3], w3[:, ci, 0:1])
                    nc.vector.scalar_tensor_tensor(
                        out=acc[:], in0=gfeat[:, ci * 3 + 1],
                        scalar=w3[:, ci, 1:2], in1=acc[:],
                        op0=Alu.mult, op1=Alu.add)
                    nc.scalar.mul(gm2[:], gfeat[:, ci * 3 + 2],
                                  w3[:, ci, 2:3])
                    nc.vector.tensor_add(acc2[:], acc[:], gm2[:])
                    nc.sync.dma_start(outS[ci * P:(ci + 1) * P, :], acc2[:])

            # ---- interleaved schedule over quarters ----
            iwx = [None] * NQ
            iwf = [None] * NQ
            for ci in range(QC):
                pass1_chunk(ci)
            iwx[0] = xblk_prep(0)
            for ci in range(QC, 2 * QC):
                pass1_chunk(ci)
            xblk_gather(0, iwx[0])
            iwx[1] = xblk_prep(1)
            for ci in range(2 * QC, 3 * QC):
                pass1_chunk(ci)
            candidates_q(0)
            xblk_gather(1, iwx[1])
            iwx[2] = xblk_prep(2)
            for ci in range(3 * QC, 4 * QC):
                pass1_chunk(ci)
            iwf[0] = feat_prep(0)
            candidates_q(1)
            xblk_gather(2, iwx[2])
            iwx[3] = xblk_prep(3)
            feat_gather(0, iwf[0])
            iwf[1] = feat_prep(1)
            candidates_q(2)
            xblk_gather(3, iwx[3])
            interp_q(0)
            feat_gather(1, iwf[1])
            iwf[2] = feat_prep(2)
            candidates_q(3)
            interp_q(1)
            feat_gather(2, iwf[2])
            iwf[3] = feat_prep(3)
            interp_q(2)
            feat_gather(3, iwf[3])
            interp_q(3)

    nc.finalize()
    return nc


def _split2(x):
    """Split fp64 array into 2 bf16 terms h+l ~ x (residual ~2^-18|x|)."""
    bf = ml_dtypes.bfloat16
    h = x.astype(bf)
    l = (x - h.astype(np.float64)).astype(bf)
    return h, l


def _host_matrices(xyz2b, xyz1b):
    """Build the K=12 bf16 contraction matrices for one batch.

    negdist[s, n] = sum_k X2[k, s] * X1[k, n]
                  ~ 2 x2_s.x1_n - |x2_s|^2 - |x1_n|^2
    """
    bf = ml_dtypes.bfloat16
    x2 = xyz2b.astype(np.float64)   # [3, S]
    x1 = xyz1b.astype(np.float64)   # [3, N]
    n2 = (x2 * x2).sum(axis=0)      # [S]
    n1 = (x1 * x1).sum(axis=0)      # [N]

    Srows, Nrows = [], []
    for c in range(3):
        qh, ql = _split2(2.0 * x2[c])
        ph, pl = _split2(x1[c])
        for a, b_ in ((qh, ph), (qh, pl), (ql, ph)):
            Srows.append(a)
            Nrows.append(b_)
    ones_s = np.ones(x2.shape[1], dtype=bf)
    ones_n = np.ones(x1.shape[1], dtype=bf)
    n2h, _ = _split2(-n2)
    Srows.append(n2h)
    Nrows.append(ones_n)
    for t in _split2(-n1):
        Srows.append(ones_s)
        Nrows.append(t)
    X2 = np.stack([np.asarray(r, dtype=bf) for r in Srows])   # [12, S]
    X1 = np.stack([np.asarray(r, dtype=bf) for r in Nrows])   # [12, N]
    return X2, X1, n2.astype(np.float32), n1.astype(np.float32)


def _prep_inputs(xyz1, xyz2, points1):
    xyz1 = np.asarray(xyz1, dtype=np.float32)
    xyz2 = np.asarray(xyz2, dtype=np.float32)
    points1 = np.asarray(points1, dtype=np.float32)
    in_maps = []
    for b in range(B):
        X2, X1, n2, n1 = _host_matrices(xyz2[b], xyz1[b])
        p1tb = np.ascontiguousarray(points1[b].T).astype(
            ml_dtypes.bfloat16)  # [N, D] bf16
        # block table: row j holds points {j + 512m}, each [2x,2y,2z,n1]
        xb = np.empty((NBLK, BPTS, 4), dtype=np.float32)
        pts = (2.0 * xyz1[b].T).reshape(BPTS, NBLK, 3)   # [m, j, 3]
        xb[:, :, 0:3] = pts.transpose(1, 0, 2)
        xb[:, :, 3] = n1.reshape(BPTS, NBLK).T
        # per-query [x, y, z, n2], laid out [p, chunk, 4]
        xq = np.empty((P, NCHUNK, 4), dtype=np.float32)
        q = xyz2[b].T.reshape(NCHUNK, P, 3)        # [chunk, p, 3]
        xq[:, :, 0:3] = q.transpose(1, 0, 2)
        xq[:, :, 3] = n2.reshape(NCHUNK, P).T
        in_maps.append({
            "x2m": X2, "x1m": X1, "p1t": p1tb,
            "xblk": xb.reshape(NBLK, BPTS * 4), "x2n": xq,
        })
    return in_maps


def _get_compiled():
    global _COMPILED
    if _COMPILED is None:
        _COMPILED = _build_bass()
    return _COMPILED


def kernel(xyz1, xyz2, points1):
    from concourse.bass_utils import run_bass_kernel_spmd

    nc = _get_compiled()
    in_maps = _prep_inputs(xyz1, xyz2, points1)
    res = run_bass_kernel_spmd(nc, in_maps, core_ids=list(range(B)))
    out = np.stack([r["outS"] for r in res.results])     # [B, S, D]
    return np.ascontiguousarray(out.transpose(0, 2, 1)).astype(np.float32)


if __name__ == "__main__":
    rng = np.random.default_rng(0)
    xyz1 = rng.standard_normal((B, 3, N), dtype=np.float32)
    xyz2 = rng.standard_normal((B, 3, S), dtype=np.float32)
    p1 = rng.standard_normal((B, D, N), dtype=np.float32)
    out = kernel(xyz1, xyz2, p1)
    print("out", out.shape, out.dtype)
